# revision 1
# baseline (speedup 1.0000x reference)
"""DeepSeek-V2 MoE MLP kernel for Trainium2, 8 NeuronCores, expert-parallel.

Sharding: 2 routed experts per core; shared-expert intermediate dim sharded
8 ways (352 cols/core); router replicated. Each core produces a partial
[2048, 2048] output (its experts' routed contributions + its shared slice);
the full output is the sum of the 8 partials.

Problem shapes (hardcoded per contract):
  T=2048 tokens, D=2048 hidden, I=1408 moe inter, E=16 experts, K=4,
  SHARED_INTER=2816, CAPACITY=768.
"""

import numpy as np

T = 2048
D = 2048
MOE_I = 1408
E = 16
TOPK = 4
SI = 2816
SI_LOC = SI // 8  # 352
CAP = 768
NCORES = 8
EXP_PER_CORE = 2
NT = T // 128     # 16 token tiles
ND = D // 128     # 16 d tiles
NI = MOE_I // 128  # 11 inter tiles
NS = CAP // 128   # 6 slot tiles per expert
# OOB marker for unfilled table rows. Must exceed every bounds_check but stay
# small enough that sentinel * row_elems fits in int32 (the DGE index math).
SENTINEL = 1.0e5


def build_nc():
    """Build the Bass/Tile program (identical for all cores)."""
    import os
    phase_limit = int(os.environ.get("MOE_PHASE_LIMIT", "4"))
    import concourse.bacc as bacc
    import concourse.bass as bass
    import concourse.mybir as mybir
    import concourse.tile as tile

    dt = mybir.dt
    f32 = dt.float32
    f32r = dt.float32r
    i32 = dt.int32
    AF = mybir.ActivationFunctionType
    ALU = mybir.AluOpType

    nc = bacc.Bacc("TRN2", target_bir_lowering=False, debug=False,
                   num_devices=NCORES)

    # ---------------- I/O ----------------
    x_d = nc.dram_tensor("x", [T, D], f32, kind="ExternalInput").ap()
    wgt_d = nc.dram_tensor("wg_t", [D, E], f32, kind="ExternalInput").ap()
    w1_d = nc.dram_tensor("w1", [EXP_PER_CORE, D, MOE_I], f32r, kind="ExternalInput").ap()
    w3_d = nc.dram_tensor("w3", [EXP_PER_CORE, D, MOE_I], f32r, kind="ExternalInput").ap()
    w2_d = nc.dram_tensor("w2", [EXP_PER_CORE, MOE_I, D], f32r, kind="ExternalInput").ap()
    ws1_d = nc.dram_tensor("ws1", [D, SI_LOC], f32r, kind="ExternalInput").ap()
    ws3_d = nc.dram_tensor("ws3", [D, SI_LOC], f32r, kind="ExternalInput").ap()
    ws2_d = nc.dram_tensor("ws2", [SI_LOC, D], f32r, kind="ExternalInput").ap()
    sel_d = nc.dram_tensor("sel", [128, EXP_PER_CORE * E], f32, kind="ExternalInput").ap()
    iota_d = nc.dram_tensor("iota128", [128, 1], f32, kind="ExternalInput").ap()
    ident_d = nc.dram_tensor("ident128", [128, 128], f32, kind="ExternalInput").ap()
    out_d = nc.dram_tensor("out", [T, D], f32, kind="ExternalOutput").ap()

    # NOTE on ordering: Tile's dependency tracking proved unreliable for
    # indirect-DMA writes followed by reads on a different DMA queue. All
    # writers/readers of table_dram and out_d therefore go through the SAME
    # queue (gpsimd / SWDGE qPoolDynamic): per-engine descriptor rings drain
    # in FIFO order, so program order implies completion order.
    with tile.TileContext(nc) as tc:
        with (
            tc.tile_pool(name="dram", bufs=1, space="DRAM") as dramp,
            tc.tile_pool(name="const", bufs=1) as constp,
            tc.tile_pool(name="xload", bufs=2) as xloadp,
            tc.tile_pool(name="ps_t", bufs=2, space="PSUM") as ps_t,
            tc.tile_pool(name="ps_mm", bufs=6, space="PSUM") as ps_mm,
            tc.tile_pool(name="small", bufs=2) as smallp,
        ):
            # ---- DRAM scratch ----
            xT_dram = dramp.tile([D, T], f32r)          # transposed x bounce
            table_dram = dramp.tile([EXP_PER_CORE * CAP, 2], f32)  # (tok, w)

            # ---- constants ----
            ident = constp.tile([128, 128], f32)
            nc.sync.dma_start(out=ident[:], in_=ident_d)
            iota_f = constp.tile([128, 1], f32)
            nc.sync.dma_start(out=iota_f[:], in_=iota_d)
            sel_sb = constp.tile([128, EXP_PER_CORE * E], f32)
            nc.sync.dma_start(out=sel_sb[:], in_=sel_d)
            wgt_sb = constp.tile([128, ND, E], f32)
            nc.sync.dma_start(out=wgt_sb[:], in_=wgt_d.rearrange("(k p) e -> p k e", p=128))

            # init table to sentinel
            sent_sb = constp.tile([128, 2], f32)
            nc.vector.memset(sent_sb[:], SENTINEL)
            n_tab_tiles = EXP_PER_CORE * CAP // 128  # 12
            for t in range(n_tab_tiles):
                nc.gpsimd.dma_start(out=table_dram[t * 128:(t + 1) * 128, :],
                                    in_=sent_sb[:])

            # ============ Phases 0-2: router + dispatch tables ============
            with (
                tc.tile_pool(name="router", bufs=1) as routerp,
                tc.tile_pool(name="xtc", bufs=3) as xtcp,
            ):
                scores = routerp.tile([128, NT, E], f32)   # softmax probs
                mask = routerp.tile([128, NT, E], f32)     # top-4 one-hot
                wsel = routerp.tile([128, NT, E], f32)     # prob * mask
                maskT = routerp.tile([E, T], f32)
                posTx = routerp.tile([E, T], f32)          # exclusive prefix count
                pos = routerp.tile([128, NT, E], f32)
                scT = routerp.tile([E, T], f32)            # raw logits transposed

                # ---- load x tiles, transpose to xT, router matmul ----
                for ti in range(NT):
                    x_tile = xloadp.tile([128, D], f32, tag="x_tile")
                    nc.sync.dma_start(out=x_tile[:], in_=x_d[ti * 128:(ti + 1) * 128, :])
                    xt_chunk = xtcp.tile([128, ND, 128], f32, tag="xt_chunk")
                    xt_r = xtcp.tile([128, ND, 128], f32r, tag="xt_r")
                    for dj in range(ND):
                        pst = ps_t.tile([128, 128], f32, space="PSUM", tag="pst")
                        nc.tensor.transpose(out=pst[:], in_=x_tile[:, dj * 128:(dj + 1) * 128],
                                            identity=ident[:])
                        nc.any.tensor_copy(xt_chunk[:, dj, :], pst[:])
                        nc.any.tensor_copy(xt_r[:, dj, :], pst[:])
                    # router logits: scT[:, ti] = wgt.T @ xT_chunk  (true fp32)
                    psc = ps_mm.tile([E, 128], f32, space="PSUM", tag="mm")
                    for dj in range(ND):
                        nc.tensor.matmul(out=psc[:], lhsT=wgt_sb[:, dj, :],
                                         rhs=xt_chunk[:, dj, :],
                                         start=(dj == 0), stop=(dj == ND - 1))
                    nc.any.tensor_copy(scT[:, ti * 128:(ti + 1) * 128], psc[:])
                    # bounce xT chunk to DRAM for the shared-expert phase
                    nc.sync.dma_start(
                        out=xT_dram[:].rearrange("(dj p) t -> p dj t", p=128)[:, :, ti * 128:(ti + 1) * 128],
                        in_=xt_r[:],
                    )

                # ---- softmax + top-4 selection ----
                for ti in range(NT):
                    pst = ps_t.tile([128, E], f32, space="PSUM", tag="pst")
                    nc.tensor.transpose(out=pst[:], in_=scT[:, ti * 128:(ti + 1) * 128],
                                        identity=ident[:E, :E])
                    logit = smallp.tile([128, E], f32, tag="logit")
                    nc.any.tensor_copy(logit[:], pst[:])
                    expv = smallp.tile([128, E], f32, tag="expv")
                    ssum = smallp.tile([128, 1], f32, tag="ssum")
                    nc.scalar.activation(expv[:], logit[:], AF.Exp, accum_out=ssum[:])
                    rsum = smallp.tile([128, 1], f32, tag="rsum")
                    nc.vector.reciprocal(rsum[:], ssum[:])
                    nc.vector.tensor_scalar_mul(scores[:, ti, :], expv[:], rsum[:, :1])
                    v8 = smallp.tile([128, 8], f32, tag="v8")
                    nc.vector.max(out=v8[:], in_=scores[:, ti, :])
                    nc.vector.tensor_scalar(
                        out=mask[:, ti, :], in0=scores[:, ti, :],
                        scalar1=v8[:, 3:4], scalar2=None, op0=ALU.is_ge)
                    nc.vector.tensor_tensor(
                        out=wsel[:, ti, :], in0=scores[:, ti, :], in1=mask[:, ti, :],
                        op=ALU.mult)
                    pst2 = ps_t.tile([E, 128], f32, space="PSUM", tag="pst")
                    nc.tensor.transpose(out=pst2[:], in_=mask[:, ti, :], identity=ident[:])
                    nc.any.tensor_copy(maskT[:, ti * 128:(ti + 1) * 128], pst2[:])

                # ---- positions via prefix scan along tokens ----
                nc.vector.tensor_tensor_scan(
                    out=posTx[:], data0=maskT[:], data1=maskT[:], initial=0.0,
                    op0=ALU.add, op1=ALU.bypass)  # inclusive cumsum
                nc.vector.tensor_tensor(out=posTx[:], in0=posTx[:], in1=maskT[:],
                                        op=ALU.subtract)  # exclusive
                for ti in range(NT):
                    pst = ps_t.tile([128, E], f32, space="PSUM", tag="pst")
                    nc.tensor.transpose(out=pst[:], in_=posTx[:, ti * 128:(ti + 1) * 128],
                                        identity=ident[:E, :E])
                    nc.any.tensor_copy(pos[:, ti, :], pst[:])

                # ---- scatter (token, weight) into dispatch table ----
                AX = mybir.AxisListType
                for ti in range(NT):
                    for j in range(EXP_PER_CORE):
                        selj = sel_sb[:, j * E:(j + 1) * E]
                        scratch = smallp.tile([128, E], f32, tag="scratch")
                        pos_j = smallp.tile([128, 1], f32, tag="pos_j")
                        nc.vector.tensor_tensor(out=scratch[:], in0=pos[:, ti, :],
                                                in1=selj, op=ALU.mult)
                        nc.vector.reduce_sum(out=pos_j[:], in_=scratch[:], axis=AX.X)
                        mask_j = smallp.tile([128, 1], f32, tag="mask_j")
                        nc.vector.tensor_tensor(out=scratch[:], in0=mask[:, ti, :],
                                                in1=selj, op=ALU.mult)
                        nc.vector.reduce_sum(out=mask_j[:], in_=scratch[:], axis=AX.X)
                        w_j = smallp.tile([128, 1], f32, tag="w_j")
                        nc.vector.tensor_tensor(out=scratch[:], in0=wsel[:, ti, :],
                                                in1=selj, op=ALU.mult)
                        nc.vector.reduce_sum(out=w_j[:], in_=scratch[:], axis=AX.X)
                        # idx = pos + j*CAP if selected else huge
                        idx_f = smallp.tile([128, 1], f32, tag="idx_f")
                        nc.vector.tensor_scalar(
                            out=idx_f[:], in0=mask_j[:], scalar1=-1.0e6, scalar2=1.0e6,
                            op0=ALU.mult, op1=ALU.add)  # 1e6*(1-mask)
                        nc.vector.tensor_tensor(out=idx_f[:], in0=idx_f[:], in1=pos_j[:],
                                                op=ALU.add)
                        if j > 0:
                            nc.vector.tensor_scalar_add(idx_f[:], idx_f[:], float(j * CAP))
                        idx_i = smallp.tile([128, 1], i32, tag="idx_i")
                        nc.vector.tensor_copy(idx_i[:], idx_f[:])
                        payload = smallp.tile([128, 2], f32, tag="payload")
                        nc.vector.tensor_scalar_add(payload[:, 0:1], iota_f[:], float(ti * 128))
                        nc.vector.tensor_copy(payload[:, 1:2], w_j[:])
                        nc.gpsimd.indirect_dma_start(
                            out=table_dram[:],
                            out_offset=bass.IndirectOffsetOnAxis(ap=idx_i[:, :1], axis=0),
                            in_=payload[:],
                            in_offset=None,
                            bounds_check=(j + 1) * CAP - 1,
                            oob_is_err=False,
                        )

            # ============ Phase 3: shared expert ============
            with (
                tc.tile_pool(name="shw", bufs=1) as shwp,
                tc.tile_pool(name="xtn", bufs=4) as xtnp,
                tc.tile_pool(name="shev", bufs=3) as shevp,
            ):
                ws1_sb = shwp.tile([128, ND, SI_LOC], f32r)
                ws3_sb = shwp.tile([128, ND, SI_LOC], f32r)
                if phase_limit >= 3:
                    nc.sync.dma_start(out=ws1_sb[:], in_=ws1_d.rearrange("(k p) i -> p k i", p=128))
                    nc.sync.dma_start(out=ws3_sb[:], in_=ws3_d.rearrange("(k p) i -> p k i", p=128))
                h_sh = shwp.tile([128, 3, T], f32r)  # [i-part, m(128/128/96), tok]
                m_sizes = [128, 128, SI_LOC - 256]  # 128,128,96
                for n in range(4 if phase_limit >= 3 else 0):  # token 512-blocks
                    gps, ups = [], []
                    for m in range(3):
                        gp = ps_mm.tile([128, 512], f32, space="PSUM", tag="mm")
                        up = ps_mm.tile([128, 512], f32, space="PSUM", tag="mm")
                        gps.append(gp)
                        ups.append(up)
                    for k in range(ND):
                        xt_nk = xtnp.tile([128, 512], f32r, tag="xt_nk")
                        nc.sync.dma_start(
                            out=xt_nk[:],
                            in_=xT_dram[k * 128:(k + 1) * 128, n * 512:(n + 1) * 512])
                        for m in range(3):
                            ms = m_sizes[m]
                            nc.tensor.matmul(out=gps[m][:ms, :],
                                             lhsT=ws1_sb[:, k, m * 128:m * 128 + ms],
                                             rhs=xt_nk[:], start=(k == 0), stop=(k == ND - 1))
                            nc.tensor.matmul(out=ups[m][:ms, :],
                                             lhsT=ws3_sb[:, k, m * 128:m * 128 + ms],
                                             rhs=xt_nk[:], start=(k == 0), stop=(k == ND - 1))
                    for m in range(3):
                        ms = m_sizes[m]
                        sg = shevp.tile([128, 512], f32, tag="sg")
                        nc.scalar.activation(sg[:ms, :], gps[m][:ms, :], AF.Sigmoid)
                        nc.vector.tensor_tensor(
                            out=sg[:ms, :], in0=sg[:ms, :], in1=gps[m][:ms, :],
                            op=ALU.mult)  # silu(g) = g * sigmoid(g)
                        nc.vector.tensor_tensor(
                            out=h_sh[:ms, m, n * 512:(n + 1) * 512], in0=sg[:ms, :],
                            in1=ups[m][:ms, :], op=ALU.mult)

                ws2_sb = shwp.tile([128, 3, D], f32r)  # k-stripes of ws2 (96 pad)
                for m in range(3 if phase_limit >= 3 else 0):
                    ms = m_sizes[m]
                    nc.sync.dma_start(out=ws2_sb[:ms, m, :], in_=ws2_d[m * 128:m * 128 + ms, :])
                for ms_i in range(NT if phase_limit >= 3 else 0):  # output token tiles
                    o_sh = shevp.tile([128, D], f32, tag="o_sh")
                    for n in range(4):
                        op = ps_mm.tile([128, 512], f32, space="PSUM", tag="mm")
                        for k in range(3):
                            ks = m_sizes[k]
                            nc.tensor.matmul(
                                out=op[:], lhsT=h_sh[:ks, k, ms_i * 128:(ms_i + 1) * 128],
                                rhs=ws2_sb[:ks, k, n * 512:(n + 1) * 512],
                                start=(k == 0), stop=(k == 2))
                        nc.any.tensor_copy(o_sh[:, n * 512:(n + 1) * 512], op[:])
                    nc.gpsimd.dma_start(out=out_d[ms_i * 128:(ms_i + 1) * 128, :], in_=o_sh[:])

            # ============ Phase 4: routed experts ============
            with (
                tc.tile_pool(name="buf", bufs=1) as bufp,
                tc.tile_pool(name="wstream", bufs=4) as wstream,
                tc.tile_pool(name="gsb", bufs=2) as gsbp,
                tc.tile_pool(name="odn", bufs=1) as odnp,
                tc.tile_pool(name="tw", bufs=2) as twp,
            ):
                for j in range(EXP_PER_CORE if phase_limit >= 4 else 0):
                    # --- read back dispatch metadata for this expert ---
                    tok_is, w_cols = [], []
                    for s in range(NS):
                        meta = smallp.tile([128, 2], f32, tag="meta")
                        nc.gpsimd.dma_start(
                            out=meta[:],
                            in_=table_dram[j * CAP + s * 128: j * CAP + (s + 1) * 128, :])
                        tok_i = twp.tile([128, 1], i32, tag=f"tok_{s}")
                        nc.vector.tensor_copy(tok_i[:], meta[:, 0:1])
                        w_col = twp.tile([128, 1], f32, tag=f"w_{s}")
                        nc.vector.tensor_copy(w_col[:], meta[:, 1:2])
                        tok_is.append(tok_i)
                        w_cols.append(w_col)

                    # --- gather token rows and transpose into bufT ---
                    bufT = bufp.tile([128, ND, CAP], f32r, tag="bufT")
                    for s in range(NS):
                        gbuf = xloadp.tile([128, D], f32, tag="x_tile")
                        nc.vector.memset(gbuf[:], 0.0)
                        nc.gpsimd.indirect_dma_start(
                            out=gbuf[:], out_offset=None,
                            in_=x_d,
                            in_offset=bass.IndirectOffsetOnAxis(ap=tok_is[s][:, :1], axis=0),
                            bounds_check=T - 1, oob_is_err=False)
                        for dj in range(ND):
                            pst = ps_t.tile([128, 128], f32, space="PSUM", tag="pst")
                            nc.tensor.transpose(out=pst[:], in_=gbuf[:, dj * 128:(dj + 1) * 128],
                                                identity=ident[:])
                            nc.any.tensor_copy(bufT[:, dj, s * 128:(s + 1) * 128], pst[:])

                    # --- up projections: hT[i, slot] ---
                    hT = bufp.tile([128, NI, CAP], f32r, tag="hT")
                    m_groups = [(0, 2), (2, 2), (4, 2), (6, 2), (8, 2), (10, 1)]
                    for (m0, mcnt) in m_groups:
                        # gate pass
                        g_sb = gsbp.tile([128, 2, CAP], f32, tag="g_sb")
                        gps = []
                        for _ in range(mcnt * 2):
                            up_ps = ps_mm.tile([128, 384], f32, space="PSUM", tag="mm")
                            gps.append(up_ps)
                        for k in range(ND):
                            wc = wstream.tile([128, 256], f32r, tag="w1c")
                            nc.sync.dma_start(
                                out=wc[:, :mcnt * 128],
                                in_=w1_d[j, k * 128:(k + 1) * 128, m0 * 128:(m0 + mcnt) * 128])
                            for mi in range(mcnt):
                                for nh in range(2):
                                    nc.tensor.matmul(
                                        out=gps[mi * 2 + nh][:],
                                        lhsT=wc[:, mi * 128:(mi + 1) * 128],
                                        rhs=bufT[:, k, nh * 384:(nh + 1) * 384],
                                        start=(k == 0), stop=(k == ND - 1))
                        for mi in range(mcnt):
                            for nh in range(2):
                                dst = g_sb[:, mi, nh * 384:(nh + 1) * 384]
                                nc.scalar.activation(
                                    dst, gps[mi * 2 + nh][:], AF.Sigmoid)
                                nc.vector.tensor_tensor(
                                    out=dst, in0=dst, in1=gps[mi * 2 + nh][:],
                                    op=ALU.mult)  # silu(g) = g * sigmoid(g)
                        # up pass
                        ups = []
                        for _ in range(mcnt * 2):
                            up_ps = ps_mm.tile([128, 384], f32, space="PSUM", tag="mm")
                            ups.append(up_ps)
                        for k in range(ND):
                            wc = wstream.tile([128, 256], f32r, tag="w3c")
                            nc.sync.dma_start(
                                out=wc[:, :mcnt * 128],
                                in_=w3_d[j, k * 128:(k + 1) * 128, m0 * 128:(m0 + mcnt) * 128])
                            for mi in range(mcnt):
                                for nh in range(2):
                                    nc.tensor.matmul(
                                        out=ups[mi * 2 + nh][:],
                                        lhsT=wc[:, mi * 128:(mi + 1) * 128],
                                        rhs=bufT[:, k, nh * 384:(nh + 1) * 384],
                                        start=(k == 0), stop=(k == ND - 1))
                        for mi in range(mcnt):
                            for nh in range(2):
                                nc.vector.tensor_tensor(
                                    out=hT[:, m0 + mi, nh * 384:(nh + 1) * 384],
                                    in0=g_sb[:, mi, nh * 384:(nh + 1) * 384],
                                    in1=ups[mi * 2 + nh][:], op=ALU.mult)

                    # --- down projection + weighted scatter-add ---
                    o_sbs = []
                    for s in range(NS):
                        o_dn = odnp.tile([128, D], f32, tag=f"o_dn_{s}")
                        o_sbs.append(o_dn)
                    for n in range(4):
                        dps = []
                        for s in range(NS):
                            dn_ps = ps_mm.tile([128, 512], f32, space="PSUM", tag="mm")
                            dps.append(dn_ps)
                        for k in range(NI):
                            wc = wstream.tile([128, 512], f32r, tag="w2c")
                            nc.sync.dma_start(
                                out=wc[:], in_=w2_d[j, k * 128:(k + 1) * 128, n * 512:(n + 1) * 512])
                            for s in range(NS):
                                nc.tensor.matmul(
                                    out=dps[s][:], lhsT=hT[:, k, s * 128:(s + 1) * 128],
                                    rhs=wc[:], start=(k == 0), stop=(k == NI - 1))
                        for s in range(NS):
                            nc.vector.tensor_scalar_mul(
                                o_sbs[s][:, n * 512:(n + 1) * 512], dps[s][:], w_cols[s][:, :1])
                    for s in range(NS):
                        nc.gpsimd.indirect_dma_start(
                            out=out_d,
                            out_offset=bass.IndirectOffsetOnAxis(ap=tok_is[s][:, :1], axis=0),
                            in_=o_sbs[s][:],
                            in_offset=None,
                            bounds_check=T - 1, oob_is_err=False,
                            compute_op=ALU.add)

    nc.compile()
    return nc


def make_in_maps(inputs):
    """Build per-core input maps from the full (unsharded) inputs."""
    x = np.ascontiguousarray(np.asarray(inputs["hidden_states"], dtype=np.float32))
    w_gate = np.asarray(inputs["w_gate"], dtype=np.float32)
    w1 = np.asarray(inputs["w1"], dtype=np.float32)
    w3 = np.asarray(inputs["w3"], dtype=np.float32)
    w2 = np.asarray(inputs["w2"], dtype=np.float32)
    ws1 = np.asarray(inputs["ws1"], dtype=np.float32)
    ws3 = np.asarray(inputs["ws3"], dtype=np.float32)
    ws2 = np.asarray(inputs["ws2"], dtype=np.float32)

    wg_t = np.ascontiguousarray(w_gate.T)  # [D, E]
    iota128 = np.arange(128, dtype=np.float32).reshape(128, 1)
    ident128 = np.eye(128, dtype=np.float32)

    in_maps = []
    for c in range(NCORES):
        e0 = EXP_PER_CORE * c
        sel = np.zeros((128, EXP_PER_CORE * E), dtype=np.float32)
        for j in range(EXP_PER_CORE):
            sel[:, j * E + e0 + j] = 1.0
        in_maps.append({
            "x": x,
            "wg_t": wg_t,
            "w1": np.ascontiguousarray(w1[e0:e0 + EXP_PER_CORE]),
            "w3": np.ascontiguousarray(w3[e0:e0 + EXP_PER_CORE]),
            "w2": np.ascontiguousarray(w2[e0:e0 + EXP_PER_CORE]),
            "ws1": np.ascontiguousarray(ws1[:, c * SI_LOC:(c + 1) * SI_LOC]),
            "ws3": np.ascontiguousarray(ws3[:, c * SI_LOC:(c + 1) * SI_LOC]),
            "ws2": np.ascontiguousarray(ws2[c * SI_LOC:(c + 1) * SI_LOC, :]),
            "sel": sel,
            "iota128": iota128,
            "ident128": ident128,
        })
    return in_maps


_NC_CACHE = None


def kernel(**inputs) -> np.ndarray:
    global _NC_CACHE
    from concourse.bass_utils import run_bass_kernel_spmd

    if _NC_CACHE is None:
        _NC_CACHE = build_nc()
    nc = _NC_CACHE
    in_maps = make_in_maps(inputs)
    res = run_bass_kernel_spmd(nc, in_maps, list(range(NCORES)))
    out = np.zeros((T, D), dtype=np.float32)
    for c in range(NCORES):
        out += res.results[c]["out"]
    return out

